# revision 16
# baseline (speedup 1.0000x reference)
"""Trainium2 Bass kernel for BlockDecomposedSSMAttention.

Math: y[b,s,:] = x[b,s,:] @ B.T @ A @ C.T   (no cross-block recurrence)
 ==>  y = x @ W  with  W = B.T @ A @ C.T    (fold params into one 1024x1024
                                             matrix -> 3x fewer FLOPs on the
                                             big tensor)

Distribution over the 8 NeuronCores:
  - x is data-parallel sharded over (batch*seq): 16384 rows -> 2048 rows/core.
  - W is computed redundantly on every core (TT = A.T @ B, then W = TT.T @ C.T),
    all in float32r (1 cycle/row at N=512, same rate as bf16, ~fp32 accuracy).
    A collective-based split-W variant measured slower and noisy: the 8-core
    AllGather costs 25-40us in trigger latency + launch-skew waits, more than
    the ~67us of redundant PE work it saves. Redundant W is deterministic.
  - main:    y_shard = x_shard @ W    (f32r matmuls, N=512 moving dim)

Host-side work is layout marshalling only (shard slicing + transposes so the
contraction dim lands on SBUF partitions); every FLOP runs on the device.
"""

import os
import sys

import numpy as np

if "/opt/trn_rl_repo" not in sys.path:
    sys.path.insert(0, "/opt/trn_rl_repo")

BATCH, SEQ, D = 4, 4096, 1024
NCORES = 8
ROWS = BATCH * SEQ            # 16384
MSH = ROWS // NCORES          # 2048 rows per core
P = 128
KT = D // P                   # 8 contraction tiles
ISL = D // NCORES             # 128-wide W row-slice per core
OC = 512                      # moving free-dim chunk (fp32 max; f32r fast dim)
NOC = D // OC                 # 2

_CACHE: dict = {}


def _build_nc():
    import concourse.mybir as mybir
    import concourse.tile as tile
    from concourse import bacc

    f32 = mybir.dt.float32
    f32r = mybir.dt.float32r

    nc = bacc.Bacc(
        "TRN2", target_bir_lowering=False, debug=False, num_devices=NCORES
    )

    xt = nc.dram_tensor("xt", [P, KT, MSH], f32, kind="ExternalInput")
    a_in = nc.dram_tensor("a_in", [D, D], f32, kind="ExternalInput")
    b_sl = nc.dram_tensor("b_sl", [D, ISL], f32, kind="ExternalInput")
    ct_in = nc.dram_tensor("ct_in", [D, D], f32, kind="ExternalInput")
    y_out = nc.dram_tensor("y_out", [MSH, D], f32, kind="ExternalOutput")

    with tile.TileContext(nc) as tc:
        with (
            tc.tile_pool(name="big", bufs=1) as big,
            tc.tile_pool(name="small", bufs=1) as small,
            tc.tile_pool(name="ycopy", bufs=6) as ycopy,
            tc.tile_pool(name="psw", bufs=3, space="PSUM") as psw,
            tc.tile_pool(name="psm", bufs=4, space="PSUM") as psm,
            tc.tile_pool(name="dram", bufs=1, space="DRAM") as dram,
        ):
            # ncfw warm-up + cross-core aligner: a tiny AllGather issued at
            # kernel start. The FIRST collective pays ~11us of ncfw startup
            # (trigger->ALGO_MESH_BEGIN); paying it here, hidden under the
            # load phase, makes the real gathers start in ~1us. It also
            # barriers the cores, collapsing launch skew before stage A.
            b_re = b_sl.ap().rearrange("(ho hp) i -> hp ho i", hp=P)
            warm_sb = small.tile([P, 4], f32)
            nc.sync.dma_start(warm_sb[:], b_re[:, 0, 0:4])
            cc_w_in = dram.tile([P, 4], f32)
            cc_w_out = dram.tile([NCORES * P, 4], f32, addr_space="Shared")
            nc.sync.dma_start(cc_w_in[:], warm_sb[:])
            nc.gpsimd.collective_compute(
                "AllGather",
                mybir.AluOpType.bypass,
                replica_groups=[list(range(NCORES))],
                ins=[cc_w_in.opt()],
                outs=[cc_w_out.opt()],
            )
            warm_back = small.tile([P, 4], f32)
            nc.sync.dma_start(warm_back[:], cc_w_out[0:P, :])

            # ---- loads, consumption-ordered ----
            b_sb = small.tile([P, KT, ISL], f32r)
            nc.sync.dma_start(b_sb[:], b_re.bitcast(f32r))
            a_sb = big.tile([P, KT, D], f32r)
            a_re = a_in.ap().rearrange("(ho hp) a -> hp ho a", hp=P)
            for j in range(KT):
                nc.sync.dma_start(
                    a_sb[:, :, j * P : (j + 1) * P],
                    a_re[:, :, j * P : (j + 1) * P].bitcast(f32r),
                )
            ct_sb = big.tile([P, KT, D], f32r)
            ct_re = ct_in.ap().rearrange("(ao ap2) o -> ap2 ao o", ap2=P)
            for oc in range(NOC):
                nc.sync.dma_start(
                    ct_sb[:, :, oc * OC : (oc + 1) * OC],
                    ct_re[:, :, oc * OC : (oc + 1) * OC].bitcast(f32r),
                )
            x_sb = big.tile([P, KT, MSH], f32r)
            for mq in range(4):
                nc.sync.dma_start(
                    x_sb[:, :, mq * OC : (mq + 1) * OC],
                    xt.ap()[:, :, mq * OC : (mq + 1) * OC].bitcast(f32r),
                )

            # ---- stage A (f32r): TT[:, i-slice(core)] = A.T @ B_slice ----
            tt_sb = small.tile([P, KT, ISL], f32r)
            for j in range(KT):
                pw = psw.tile([P, OC], f32, tag="pwb")
                for h in range(KT):
                    nc.tensor.matmul(
                        pw[:, :ISL],
                        a_sb[:, h, j * P : (j + 1) * P],
                        b_sb[:, h, :],
                        start=(h == 0),
                        stop=(h == KT - 1),
                    )
                nc.vector.tensor_copy(tt_sb[:, j, :], pw[:, :ISL])

            # ---- stage B (f32r) per half: W_slice -> AllGather half ----
            w_loc = small.tile([P, D], f32)
            cc_in0 = dram.tile([P, OC], f32)
            cc_in1 = dram.tile([P, OC], f32)
            cc_out0 = dram.tile([NCORES * P, OC], f32, addr_space="Shared")
            cc_out1 = dram.tile([NCORES * P, OC], f32, addr_space="Shared")
            cc_ins = [cc_in0, cc_in1]
            cc_outs = [cc_out0, cc_out1]
            w_sb = big.tile([P, KT, D], f32r)
            for oc in range(NOC):
                pb = psw.tile([P, OC], f32, tag="pwb")
                for j in range(KT):
                    nc.tensor.matmul(
                        pb[:],
                        tt_sb[:, j, :],
                        ct_sb[:, j, oc * OC : (oc + 1) * OC],
                        start=(j == 0),
                        stop=(j == KT - 1),
                    )
                nc.vector.tensor_copy(w_loc[:, oc * OC : (oc + 1) * OC], pb[:])
                nc.sync.dma_start(cc_ins[oc][:], w_loc[:, oc * OC : (oc + 1) * OC])
                nc.gpsimd.collective_compute(
                    "AllGather",
                    mybir.AluOpType.bypass,
                    replica_groups=[list(range(NCORES))],
                    ins=[cc_ins[oc].opt()],
                    outs=[cc_outs[oc].opt()],
                )
                src_re = cc_outs[oc].rearrange("(ko kp) o -> kp ko o", kp=P)
                for kq in range(4):
                    nc.sync.dma_start(
                        w_sb[:, kq * 2 : (kq + 1) * 2, oc * OC : (oc + 1) * OC],
                        src_re[:, kq * 2 : (kq + 1) * 2, :].bitcast(f32r),
                    )

            # ---- main loop (f32r), oc-major so oc=0 needs only gather 0 ----
            for oc in range(NOC):
                for mt in range(MSH // P):
                    pm = psm.tile([P, OC], f32)
                    for k in range(KT):
                        nc.tensor.matmul(
                            pm[:],
                            x_sb[:, k, mt * P : (mt + 1) * P],
                            w_sb[:, k, oc * OC : (oc + 1) * OC],
                            start=(k == 0),
                            stop=(k == KT - 1),
                        )
                    yt = ycopy.tile([P, OC], f32)
                    nc.vector.tensor_copy(yt[:], pm[:])
                    nc.gpsimd.dma_start(
                        y_out.ap()[mt * P : (mt + 1) * P, oc * OC : (oc + 1) * OC],
                        yt[:],
                    )

    nc.compile()
    return nc


def _get_nc():
    if "nc" not in _CACHE:
        _CACHE["nc"] = _build_nc()
    return _CACHE["nc"]


def _make_in_maps(x, A, B, C):
    x2 = np.ascontiguousarray(x, dtype=np.float32).reshape(ROWS, D)
    ct = np.ascontiguousarray(C.T, dtype=np.float32)
    a_full = np.ascontiguousarray(A, dtype=np.float32)
    in_maps = []
    for c in range(NCORES):
        shard = x2[c * MSH : (c + 1) * MSH]  # [MSH, D]
        # [kp, ko, m] with element (kp,ko,m) = shard[m, ko*128+kp]
        xtc = np.ascontiguousarray(shard.reshape(MSH, KT, P).transpose(2, 1, 0))
        bsl = np.ascontiguousarray(B[:, c * ISL : (c + 1) * ISL], dtype=np.float32)
        in_maps.append({"xt": xtc, "a_in": a_full, "b_sl": bsl, "ct_in": ct})
    return in_maps


def _install_ntff_hook():
    """The agent image's ``antenv`` lacks ``axon_hooks``; recreate it and
    register the ctypes-based NTFF profile hook (same as trn_boot's
    ``_ntff_profile_via_ctypes``) so ``trace=True`` yields exec_time_ns."""
    import contextlib
    import ctypes
    import types

    if "antenv.axon_hooks" in sys.modules:
        return True
    so_path = "/opt/axon/libaxon_pjrt.so"
    if not os.path.exists(so_path):
        return False
    lib = ctypes.CDLL(so_path)
    if not hasattr(lib, "axon_start_nrt_profile"):
        return False
    lib.axon_start_nrt_profile.argtypes = [
        ctypes.POINTER(ctypes.c_int64),
        ctypes.c_size_t,
    ]
    lib.axon_start_nrt_profile.restype = ctypes.c_int64
    lib.axon_stop_nrt_profile.argtypes = [ctypes.c_char_p]
    lib.axon_stop_nrt_profile.restype = ctypes.c_int64

    @contextlib.contextmanager
    def _hook(output_dir, device_ids):
        import jax

        jax.devices()
        if device_ids:
            ids = (ctypes.c_int64 * len(device_ids))(*device_ids)
            rc = lib.axon_start_nrt_profile(ids, len(device_ids))
        else:
            rc = lib.axon_start_nrt_profile(None, 0)
        if rc != 0:
            raise RuntimeError(f"axon_start_nrt_profile rc={rc}")
        try:
            yield
        finally:
            n = lib.axon_stop_nrt_profile(str(output_dir).encode())
            print(f"ntff profile: {n} file(s) written to {output_dir}")

    mod = types.ModuleType("antenv.axon_hooks")
    _state = {"hook": _hook}
    mod.set_axon_ntff_profile_hook = lambda h: _state.__setitem__("hook", h)
    mod.get_axon_ntff_profile_hook = lambda: _state["hook"]
    sys.modules["antenv.axon_hooks"] = mod
    import antenv

    antenv.axon_hooks = mod
    return True


def run(x, A, B, C, trace=False):
    """Run on hardware; returns (y_full, exec_time_ns_or_None)."""
    from concourse import bass_utils
    from concourse.bass_interp import get_hw_module

    if trace and not _install_ntff_hook():
        trace = False
    if trace:
        # upload_artifacts pushes the NEFF dir to a remote bucket; in this
        # sandbox that can fail AFTER a successful run, losing the results.
        # Degrade to the local path. (Only touches the tracing dev path.)
        if not getattr(bass_utils.upload_artifacts, "_safe", False):
            _orig_upload = bass_utils.upload_artifacts

            def _safe_upload(tmpdir):
                try:
                    return _orig_upload(tmpdir)
                except Exception as e:
                    print(f"upload_artifacts skipped ({type(e).__name__}): {e}")
                    return str(tmpdir)

            _safe_upload._safe = True
            bass_utils.upload_artifacts = _safe_upload

    nc = _get_nc()
    in_maps = _make_in_maps(x, A, B, C)

    old_m = nc.m
    nc.m = get_hw_module(nc.m)
    try:
        res = bass_utils.run_bass_kernel_spmd(
            nc, in_maps, core_ids=list(range(NCORES)), trace=trace
        )
    finally:
        nc.m = old_m

    y = np.concatenate(
        [res.results[c]["y_out"] for c in range(NCORES)], axis=0
    ).reshape(BATCH, SEQ, D)
    return y, res.exec_time_ns


def kernel(x, A, B, C):
    y, _ = run(x, A, B, C, trace=False)
    return y


# revision 17
# speedup vs baseline: 1.6045x; 1.6045x over previous
"""Trainium2 Bass kernel for BlockDecomposedSSMAttention.

Math: y[b,s,:] = x[b,s,:] @ B.T @ A @ C.T   (no cross-block recurrence)
 ==>  y = x @ W  with  W = B.T @ A @ C.T    (fold params into one 1024x1024
                                             matrix -> 3x fewer FLOPs on the
                                             big tensor)

Distribution over the 8 NeuronCores:
  - x is data-parallel sharded over (batch*seq): 16384 rows -> 2048 rows/core.
  - W is computed redundantly on every core (TT = A.T @ B, then W = TT.T @ C.T),
    all in float32r (1 cycle/row at N=512, same rate as bf16, ~fp32 accuracy).
    A collective-based split-W variant measured slower and noisy: the 8-core
    AllGather costs 25-40us in trigger latency + launch-skew waits, more than
    the ~67us of redundant PE work it saves. Redundant W is deterministic.
  - main:    y_shard = x_shard @ W    (f32r matmuls, N=512 moving dim)

Host-side work is layout marshalling only (shard slicing + transposes so the
contraction dim lands on SBUF partitions); every FLOP runs on the device.
"""

import os
import sys

import numpy as np

if "/opt/trn_rl_repo" not in sys.path:
    sys.path.insert(0, "/opt/trn_rl_repo")

BATCH, SEQ, D = 4, 4096, 1024
NCORES = 8
ROWS = BATCH * SEQ            # 16384
MSH = ROWS // NCORES          # 2048 rows per core
P = 128
KT = D // P                   # 8 contraction tiles
OC = 512                      # moving free-dim chunk (fp32 max; f32r fast dim)
NOC = D // OC                 # 2
MH = MSH // 2                 # x half (SBUF staging)

_CACHE: dict = {}


def _build_nc():
    import concourse.mybir as mybir
    import concourse.tile as tile
    from concourse import bacc

    f32 = mybir.dt.float32
    f32r = mybir.dt.float32r

    nc = bacc.Bacc(
        "TRN2", target_bir_lowering=False, debug=False, num_devices=NCORES
    )

    # I/O (per-core shards; layouts chosen so every matmul operand is a
    # natural [contraction-on-partitions] SBUF load)
    xt = nc.dram_tensor("xt", [P, KT, MSH], f32, kind="ExternalInput")
    a_in = nc.dram_tensor("a_in", [D, D], f32, kind="ExternalInput")
    b_in = nc.dram_tensor("b_in", [D, D], f32, kind="ExternalInput")
    ct_in = nc.dram_tensor("ct_in", [D, D], f32, kind="ExternalInput")
    y_out = nc.dram_tensor("y_out", [MSH, D], f32, kind="ExternalOutput")

    with tile.TileContext(nc) as tc:
        with (
            tc.tile_pool(name="big", bufs=1) as big,
            tc.tile_pool(name="ycopy", bufs=6) as ycopy,
            tc.tile_pool(name="psw", bufs=4, space="PSUM") as psw,
            tc.tile_pool(name="psm", bufs=4, space="PSUM") as psm,
        ):
            # ---- SBUF loads, ordered by first consumption:
            # stage A group (j, ic) consumes a-column-chunk j (all h) and
            # b h-chunks progressively, so chunk a by columns and b by h.
            a_sb, free_a = tc.tile([P, KT, D], f32r, name="a_sb")
            b_sb, free_b = tc.tile([P, KT, D], f32r, name="b_sb")
            a_re = a_in.ap().rearrange("(ho hp) a -> hp ho a", hp=P)
            b_re = b_in.ap().rearrange("(ho hp) i -> hp ho i", hp=P)
            nc.sync.dma_start(
                a_sb[:, :, 0:P], a_re[:, :, 0:P].bitcast(f32r)
            )
            # b chunked (h-pair x ic-half) in exact stage-A consumption order
            for hq in range(4):
                nc.sync.dma_start(
                    b_sb[:, hq * 2 : (hq + 1) * 2, 0:OC],
                    b_re[:, hq * 2 : (hq + 1) * 2, 0:OC].bitcast(f32r),
                )
            for j in range(1, KT):
                nc.sync.dma_start(
                    a_sb[:, :, j * P : (j + 1) * P],
                    a_re[:, :, j * P : (j + 1) * P].bitcast(f32r),
                )
            for hq in range(4):
                nc.sync.dma_start(
                    b_sb[:, hq * 2 : (hq + 1) * 2, OC:D],
                    b_re[:, hq * 2 : (hq + 1) * 2, OC:D].bitcast(f32r),
                )
            ct_sb = big.tile([P, KT, D], f32r)
            for jq in range(2):
                nc.sync.dma_start(
                    ct_sb[:, jq * 4 : (jq + 1) * 4, :],
                    ct_in.ap().rearrange("(ao ap2) o -> ap2 ao o", ap2=P)[
                        :, jq * 4 : (jq + 1) * 4, :
                    ].bitcast(f32r),
                )
            x0_sb = big.tile([P, KT, MH], f32r)
            for mq in range(2):
                nc.sync.dma_start(
                    x0_sb[:, :, mq * OC : (mq + 1) * OC],
                    xt.ap()[:, :, mq * OC : (mq + 1) * OC].bitcast(f32r),
                )

            # ---- stage A (f32r, N=512): TT = A.T @ B, full, per core ----
            tt_sb = big.tile([P, KT, D], f32r)  # [a_p, a_tile j, i]
            for ic in range(NOC):
                for j in range(KT):
                    pw = psw.tile([P, OC], f32)
                    for h in range(KT):
                        nc.tensor.matmul(
                            pw[:],
                            a_sb[:, h, j * P : (j + 1) * P],
                            b_sb[:, h, ic * OC : (ic + 1) * OC],
                            start=(h == 0),
                            stop=(h == KT - 1),
                        )
                    nc.vector.tensor_copy(
                        tt_sb[:, j, ic * OC : (ic + 1) * OC], pw[:]
                    )
            free_b()
            free_a()

            # second x half after a/b freed (SBUF headroom)
            x1_sb, _free_x1 = tc.tile([P, KT, MH], f32r, name="x1_sb")
            for mq in range(2):
                nc.sync.dma_start(
                    x1_sb[:, :, mq * OC : (mq + 1) * OC],
                    xt.ap()[:, :, MH + mq * OC : MH + (mq + 1) * OC].bitcast(f32r),
                )

            # ---- stage B (f32r): W = TT.T @ C.T, full, per core ----
            w_sb, _free_w = tc.tile([P, KT, D], f32r, name="w_sb")  # [i_p, i_tile t, o]
            for oc in range(NOC):
                for t in range(KT):
                    pw = psw.tile([P, OC], f32)
                    for j in range(KT):
                        nc.tensor.matmul(
                            pw[:],
                            tt_sb[:, j, t * P : (t + 1) * P],
                            ct_sb[:, j, oc * OC : (oc + 1) * OC],
                            start=(j == 0),
                            stop=(j == KT - 1),
                        )
                    nc.vector.tensor_copy(
                        w_sb[:, t, oc * OC : (oc + 1) * OC], pw[:]
                    )

            # ---- main loop (f32r): y_shard = x_shard @ W ----
            for oc in range(NOC):
                for mt in range(MSH // P):
                    xh = x0_sb if mt < KT else x1_sb
                    ms = (mt % KT) * P
                    pm = psm.tile([P, OC], f32)
                    for k in range(KT):
                        nc.tensor.matmul(
                            pm[:],
                            xh[:, k, ms : ms + P],
                            w_sb[:, k, oc * OC : (oc + 1) * OC],
                            start=(k == 0),
                            stop=(k == KT - 1),
                        )
                    yt = ycopy.tile([P, OC], f32)
                    nc.vector.tensor_copy(yt[:], pm[:])
                    nc.gpsimd.dma_start(
                        y_out.ap()[mt * P : (mt + 1) * P, oc * OC : (oc + 1) * OC],
                        yt[:],
                    )

            _free_w()
            _free_x1()

    nc.compile()
    return nc


def _get_nc():
    if "nc" not in _CACHE:
        _CACHE["nc"] = _build_nc()
    return _CACHE["nc"]


def _make_in_maps(x, A, B, C):
    x2 = np.ascontiguousarray(x, dtype=np.float32).reshape(ROWS, D)
    ct = np.ascontiguousarray(C.T, dtype=np.float32)
    a_full = np.ascontiguousarray(A, dtype=np.float32)
    b_full = np.ascontiguousarray(B, dtype=np.float32)
    in_maps = []
    for c in range(NCORES):
        shard = x2[c * MSH : (c + 1) * MSH]  # [MSH, D]
        # [kp, ko, m] with element (kp,ko,m) = shard[m, ko*128+kp]
        xtc = np.ascontiguousarray(shard.reshape(MSH, KT, P).transpose(2, 1, 0))
        in_maps.append({"xt": xtc, "a_in": a_full, "b_in": b_full, "ct_in": ct})
    return in_maps


def _install_ntff_hook():
    """The agent image's ``antenv`` lacks ``axon_hooks``; recreate it and
    register the ctypes-based NTFF profile hook (same as trn_boot's
    ``_ntff_profile_via_ctypes``) so ``trace=True`` yields exec_time_ns."""
    import contextlib
    import ctypes
    import types

    if "antenv.axon_hooks" in sys.modules:
        return True
    so_path = "/opt/axon/libaxon_pjrt.so"
    if not os.path.exists(so_path):
        return False
    lib = ctypes.CDLL(so_path)
    if not hasattr(lib, "axon_start_nrt_profile"):
        return False
    lib.axon_start_nrt_profile.argtypes = [
        ctypes.POINTER(ctypes.c_int64),
        ctypes.c_size_t,
    ]
    lib.axon_start_nrt_profile.restype = ctypes.c_int64
    lib.axon_stop_nrt_profile.argtypes = [ctypes.c_char_p]
    lib.axon_stop_nrt_profile.restype = ctypes.c_int64

    @contextlib.contextmanager
    def _hook(output_dir, device_ids):
        import jax

        jax.devices()
        if device_ids:
            ids = (ctypes.c_int64 * len(device_ids))(*device_ids)
            rc = lib.axon_start_nrt_profile(ids, len(device_ids))
        else:
            rc = lib.axon_start_nrt_profile(None, 0)
        if rc != 0:
            raise RuntimeError(f"axon_start_nrt_profile rc={rc}")
        try:
            yield
        finally:
            n = lib.axon_stop_nrt_profile(str(output_dir).encode())
            print(f"ntff profile: {n} file(s) written to {output_dir}")

    mod = types.ModuleType("antenv.axon_hooks")
    _state = {"hook": _hook}
    mod.set_axon_ntff_profile_hook = lambda h: _state.__setitem__("hook", h)
    mod.get_axon_ntff_profile_hook = lambda: _state["hook"]
    sys.modules["antenv.axon_hooks"] = mod
    import antenv

    antenv.axon_hooks = mod
    return True


def run(x, A, B, C, trace=False):
    """Run on hardware; returns (y_full, exec_time_ns_or_None)."""
    from concourse import bass_utils
    from concourse.bass_interp import get_hw_module

    if trace and not _install_ntff_hook():
        trace = False
    if trace:
        # upload_artifacts pushes the NEFF dir to a remote bucket; in this
        # sandbox that can fail AFTER a successful run, losing the results.
        # Degrade to the local path. (Only touches the tracing dev path.)
        if not getattr(bass_utils.upload_artifacts, "_safe", False):
            _orig_upload = bass_utils.upload_artifacts

            def _safe_upload(tmpdir):
                try:
                    return _orig_upload(tmpdir)
                except Exception as e:
                    print(f"upload_artifacts skipped ({type(e).__name__}): {e}")
                    return str(tmpdir)

            _safe_upload._safe = True
            bass_utils.upload_artifacts = _safe_upload

    nc = _get_nc()
    in_maps = _make_in_maps(x, A, B, C)

    old_m = nc.m
    nc.m = get_hw_module(nc.m)
    try:
        res = bass_utils.run_bass_kernel_spmd(
            nc, in_maps, core_ids=list(range(NCORES)), trace=trace
        )
    finally:
        nc.m = old_m

    y = np.concatenate(
        [res.results[c]["y_out"] for c in range(NCORES)], axis=0
    ).reshape(BATCH, SEQ, D)
    return y, res.exec_time_ns


def kernel(x, A, B, C):
    y, _ = run(x, A, B, C, trace=False)
    return y
